# revision 27
# baseline (speedup 1.0000x reference)
"""Trainium2 Bass kernel for nn_MultiHeadAttention_42752104464925.

Multi-head attention (VITS-style) with windowed relative position embeddings
(window=4, heads_share=True).

Math notes
----------
With L=1024, WIN=4, the relative-key logits term rel_to_abs(q_scaled @ rel_k^T)
is a 9-diagonal band:   scores[t,s] += q_scaled[t] . emb_k[s-t+4]   (|s-t|<=4)
and the relative-value term is:
  out[t] += sum_j p[t, t+j-4] * emb_v[j]   (0 <= t+j-4 < L)

Sharding: 8 cores = 4 batches x 2 head-groups (6 heads each). Each core
computes QKV for its 384 channels, attention for its 6 heads, and a partial
output projection Wo[:, slice] @ merged. Host sums the two partials per batch.

Device layout per head: scores are computed TRANSPOSED (S^T[s,t], s on
partitions) so that A@V needs no transpose of the attention weights; the
softmax denominator L[t] (a partition-dim sum) comes for free from a ones
column appended to the V^T stationary operand (M=66).

Band application (the shear between diagonal and absolute coordinates) is
DMA-descriptor-bound if done naively, so both bands are engineered for
coarse descriptor runs:

* band_k: Rt[t,j] values for all 6 heads are written head-interleaved into a
  per-chunk skewed DRAM region (24B runs, one per (t,j)), laid out so the
  readback of the full [128, 136] band window for ALL 6 heads is a single
  per-partition-contiguous read (3264B runs). The zero padding the windows
  rely on is supplied as host-side zero input tensors.
* band_v: the 9 diagonals of exp(S) are read back from the es window staging
  as a compact [128, 9]-per-head tile (36B contiguous runs), transposed and
  row-permuted on TensorE, shear-aligned with 9 SBUF->SBUF DMAs, and folded
  into the A@V PSUM accumulation with small K=18 matmuls against a
  host-precomputed embedding matrix.

Scheduling: scores/exp are double-buffered at 512-column granularity (two
half-chunk PSUM tiles alternate) so TensorE and ScalarE pipeline; DMAs are
spread across the sync/scalar HWDGE queues and the gpsimd SWDGE queue to
avoid serializing on one sequencer.
"""

import math

import numpy as np

import concourse.bacc as bacc
import concourse.bass as bass
import concourse.mybir as mybir
import concourse.tile as tile
from concourse import library_config
from concourse.bass_utils import run_bass_kernel_spmd

# Problem constants (hardcoded per harness contract).
B, C, T, H, KC, WIN = 4, 768, 1024, 12, 64, 4
HL = 6            # heads per core
CL = HL * KC      # 384 local channels
NSUB = C // 128   # 6 k-subtiles over C
LSUB = CL // 128  # 3 subtiles over CL
NCH = T // 128    # 8 s-chunks
NB = 9            # band width (2*WIN+1)
WQ = 136          # band window width per 128-chunk (128 + 2*WIN)
# band_k skewed staging: region element addr = RK_GUARD + RK_ROW*p + 6*f + h
# holds W[p, f, h] = Rt_h[t0+f, p-f+8]; rows are 816 elements used of RK_ROW.
RK_ROW = 6 * 144
RK_GUARD = 6912
RK_LEN = RK_GUARD + RK_ROW * WQ
W2_ROWS, W2_COLS = 144, 136  # padded expS window staging (8 zero rows each end)
W2_REG = W2_ROWS * W2_COLS   # one head's region; a pair shares one tensor

F32 = mybir.dt.float32
AF = mybir.ActivationFunctionType
ALU = mybir.AluOpType

# Matmul input dtype. float32r streams at full PE rate (1 cyc/row for N>=256)
# with fp32 storage; plain float32 runs at 1/4 rate.
MM_DT = mybir.dt.float32r

EN_BANDK = True    # band_k window read + add into scores
EN_BANDV = True    # band_v pdw gather + transpose + matmul accumulation
EN_NORM = True     # softmax normalization


def _mm(x):
    return x if MM_DT == F32 else x.bitcast(MM_DT)


def _f32(x):
    return x if MM_DT == F32 else x.bitcast(F32)


def _raw(t_ap, off, dims):
    """Raw element-offset AP into (the tensor behind) an AP."""
    return bass.AP(tensor=t_ap.tensor, offset=t_ap.offset + off, ap=dims)


def _chunk_window(c):
    """Clipped t-window [t_lo, t_lo+w) for s-chunk c; q0 = offset into the
    unclipped 136-wide window starting at t0 = 128c - 4."""
    t0 = 128 * c - WIN
    t_lo = max(0, t0)
    q0 = t_lo - t0
    w = min(T, t0 + WQ) - t_lo
    return t_lo, q0, w


def _half_segments(c, n):
    """Absolute-t segments of chunk c's band window inside half n."""
    t_lo, q0, w = _chunk_window(c)
    a = max(t_lo, 512 * n)
    b = min(t_lo + w, 512 * (n + 1))
    return [(a, b)] if a < b else []


def _bandv_segments():
    """(c, a, b) absolute-t segments of each chunk's band window, split at
    PSUM bank (512) boundaries."""
    segs = []
    for c in range(NCH):
        t0 = 128 * c - WIN
        a, b = max(t0, 0), min(t0 + WQ, T)
        cuts = [a] + [x for x in (512,) if a < x < b] + [b]
        for k in range(len(cuts) - 1):
            segs.append((c, cuts[k], cuts[k + 1]))
    return segs


def build_program():
    nc = bacc.Bacc("TRN2", target_bir_lowering=False, debug=False,
                   enable_asserts=True)

    # ---- I/O ----
    xb = nc.dram_tensor("xb", [C, T], F32, kind="ExternalInput")
    cb = nc.dram_tensor("cb", [C, T], F32, kind="ExternalInput")
    wqt = nc.dram_tensor("wqt", [C, CL], F32, kind="ExternalInput")
    wkt = nc.dram_tensor("wkt", [C, CL], F32, kind="ExternalInput")
    wvt = nc.dram_tensor("wvt", [C, CL], F32, kind="ExternalInput")
    wot = nc.dram_tensor("wot", [CL, C], F32, kind="ExternalInput")
    bq2 = nc.dram_tensor("bq2", [128, LSUB], F32, kind="ExternalInput")
    bk2 = nc.dram_tensor("bk2", [128, LSUB], F32, kind="ExternalInput")
    bvr = nc.dram_tensor("bvr", [128, CL], F32, kind="ExternalInput")
    ekt18d = nc.dram_tensor("ekt18", [128, 2 * NB], F32, kind="ExternalInput")
    ev18d = nc.dram_tensor("ev18", [2 * NB, 128], F32, kind="ExternalInput")
    i128d = nc.dram_tensor("i128", [128, 128], F32, kind="ExternalInput")
    permd = nc.dram_tensor("perm18", [2 * NB, 2 * NB], F32,
                           kind="ExternalInput")
    z18d = nc.dram_tensor("z18", [2 * NB, NCH, WQ], F32,
                          kind="ExternalInput")
    ones8 = nc.dram_tensor("ones8", [128, NCH, 2], F32, kind="ExternalInput")
    # zero-padded staging buffers (host supplies zeros; device writes data)
    rk = [nc.dram_tensor(f"rk{c}", [RK_LEN], F32, kind="ExternalInput")
          for c in range(NCH)]
    w2 = [nc.dram_tensor(f"w2_{i}", [2 * W2_REG], F32, kind="ExternalInput")
          for i in range((HL // 2) * NCH)]
    outp = nc.dram_tensor("outp", [C, T], F32, kind="ExternalOutput")

    with tile.TileContext(nc) as tc:
        nc.gpsimd.load_library(library_config.attn)
        with tc.tile_pool(name="persist", bufs=1) as pp:
            # persistent SBUF
            q_sb = pp.tile([128, LSUB, T], MM_DT, tag="q_sb")
            k_sb = pp.tile([128, LSUB, T], MM_DT, tag="k_sb")
            vt = [pp.tile([128, NCH, KC + 2], MM_DT, tag=f"vt{h}", name=f"vt{h}")
                  for h in range(HL)]
            wo_sb = pp.tile([128, LSUB, C], MM_DT, tag="wo_sb")
            merged = pp.tile([128, LSUB, T], MM_DT, tag="merged")
            ekt_sb = pp.tile([128, 2 * NB], MM_DT, tag="ekt_sb")
            ev18_sb = pp.tile([2 * NB, 128], MM_DT, tag="ev18_sb")
            i128_sb = pp.tile([128, 128], MM_DT, tag="i128_sb")
            perm_sb = pp.tile([2 * NB, 2 * NB], MM_DT, tag="perm_sb")
            bq_sb = pp.tile([128, LSUB], F32, tag="bq_sb")
            bk_sb = pp.tile([128, LSUB], F32, tag="bk_sb")
            bv_sb = pp.tile([128, CL], F32, tag="bv_sb")
            # band_k windows for all chunks/heads: [p, c, f, h]
            wt6 = pp.tile([128, NCH, WQ, HL], F32, tag="wt6")

            # ---------------- Phase A: projections ----------------
            with tc.tile_pool(name="pa", bufs=1) as pa, \
                 tc.tile_pool(name="pa_ps", bufs=3, space="PSUM") as pa_ps, \
                 tc.tile_pool(name="pa_ps2", bufs=2, space="PSUM") as pa_ps2, \
                 tc.tile_pool(name="pa_ps3", bufs=2, space="PSUM") as pa_ps3:
                x_sb = pa.tile([128, NSUB, T], MM_DT, tag="x_sb")
                c_sb = pa.tile([128, NSUB, T], MM_DT, tag="c_sb")
                wq_sb = pa.tile([128, NSUB, CL], MM_DT, tag="wq_sb")
                wk_sb = pa.tile([128, NSUB, CL], MM_DT, tag="wk_sb")
                wv_sb = pa.tile([128, NSUB, CL], MM_DT, tag="wv_sb")
                # Rt staging, head-interleaved: [t_part, c, j, h]
                rts = pa.tile([128, NCH, NB, HL], F32, tag="rts")

                # whole-tensor loads (per-DMA fixed cost dominates small
                # transfers), ordered so the Q matmuls can start earliest
                nc.sync.dma_start(wq_sb[:], _mm(wqt.ap().rearrange(
                    "(s p) m -> p s m", p=128)))
                nc.sync.dma_start(x_sb[:], _mm(xb.ap().rearrange(
                    "(s p) t -> p s t", p=128)))
                nc.scalar.dma_start(wk_sb[:], _mm(wkt.ap().rearrange(
                    "(s p) m -> p s m", p=128)))
                nc.scalar.dma_start(c_sb[:], _mm(cb.ap().rearrange(
                    "(s p) t -> p s t", p=128)))
                nc.sync.dma_start(wv_sb[:], _mm(wvt.ap().rearrange(
                    "(s p) m -> p s m", p=128)))
                # persist-tile loads (needed later; keep off the hot path)
                nc.scalar.dma_start(wo_sb[:], _mm(wot.ap().rearrange(
                    "(s p) m -> p s m", p=128)))
                nc.scalar.dma_start(ekt_sb[:], _mm(ekt18d.ap()))
                nc.scalar.dma_start(ev18_sb[:], _mm(ev18d.ap()))
                nc.scalar.dma_start(i128_sb[:], _mm(i128d.ap()))
                nc.scalar.dma_start(perm_sb[:], _mm(permd.ap()))
                nc.scalar.dma_start(bq_sb[:], bq2.ap())
                nc.scalar.dma_start(bk_sb[:], bk2.ap())
                nc.scalar.dma_start(bv_sb[:], bvr.ap())
                for h in range(HL):
                    nc.scalar.dma_start(vt[h][:, :, KC:KC + 2],
                                        _mm(ones8.ap()))

                # Q and K: out[dl, t] = sum_c W*T[c, dl] * x[c, t]  (+bias)
                for dst, wsb, src, bias in ((q_sb, wq_sb, x_sb, bq_sb),
                                            (k_sb, wk_sb, c_sb, bk_sb)):
                    for m in range(LSUB):
                        for n in range(2):
                            ps = pa_ps.tile([128, 512], F32, tag="qk_ps")
                            for k in range(NSUB):
                                nc.tensor.matmul(
                                    ps[:],
                                    wsb[:, k, 128 * m:128 * (m + 1)],
                                    src[:, k, 512 * n:512 * (n + 1)],
                                    start=(k == 0), stop=(k == NSUB - 1))
                            # fused copy+bias on ACT (idle in phase A)
                            nc.scalar.activation(
                                dst[:, m, 512 * n:512 * (n + 1)], ps[:],
                                AF.Identity, bias=bias[:, m:m + 1])

                # Rt[t, j] for the head pair of subtile `sub` in one matmul:
                # stationary q-chunk [128, 128], moving block-diagonal
                # ekt18 [128, 18]  ->  out[t, 9*hl + j]
                for sub in range(LSUB):
                    for c in range(NCH):
                        rt_ps = pa_ps3.tile([128, 2 * NB], F32, tag="rt_ps")
                        nc.tensor.matmul(
                            rt_ps[:],
                            q_sb[:, sub, 128 * c:128 * (c + 1)],
                            ekt_sb[:],
                            start=True, stop=True)
                        nc.vector.tensor_copy(
                            rts[:, c, :, 2 * sub:2 * sub + 2].transpose(
                                [0, 2, 1]),
                            rt_ps[:].rearrange("p (hl j) -> p hl j", hl=2))

                # band_k staging: shear-write Rt into per-chunk skewed regions
                # (24B runs), then read each chunk's full 6-head window back
                # with per-partition-contiguous 3264B runs.
                if EN_BANDK:
                    for c in range(NCH):
                        nc.sync.dma_start(
                            _raw(rk[c].ap(), RK_GUARD - 864 * 4 + 24,
                                 [[870, 128], [864, NB], [1, HL]]),
                            rts[:, c, :, :])
                        if c > 0:
                            nc.sync.dma_start(
                                _raw(rk[c].ap(), 0,
                                     [[870, 4], [864, NB], [1, HL]]),
                                rts[124:128, c - 1, :, :])
                        if c < NCH - 1:
                            nc.sync.dma_start(
                                _raw(rk[c].ap(), RK_GUARD + 864 * 124 + 792,
                                     [[870, 4], [864, NB], [1, HL]]),
                                rts[0:4, c + 1, :, :])
                        nc.sync.dma_start(
                            wt6[:, c, :, :],
                            _raw(rk[c].ap(), RK_GUARD,
                                 [[RK_ROW, 128], [1, 6 * WQ]]))

                # V^T: out[s, dl] = sum_c c_b[c, s] * WvT[c, dl] (+bias),
                # written per head into [128, NCH, 66] tiles, col 64 = ones.
                for c in range(NCH):
                    vt_ps = pa_ps2.tile([128, CL], F32, tag="vt_ps")
                    for k in range(NSUB):
                        nc.tensor.matmul(
                            vt_ps[:],
                            c_sb[:, k, 128 * c:128 * (c + 1)],
                            wv_sb[:, k, :],
                            start=(k == 0), stop=(k == NSUB - 1))
                    for h in range(HL):
                        nc.vector.tensor_tensor(
                            vt[h][:, c, 0:KC], vt_ps[:, KC * h:KC * (h + 1)],
                            bv_sb[:, KC * h:KC * (h + 1)], ALU.add)

            # ---------------- Phase B: attention ----------------
            segs = _bandv_segments()
            last_half = {}
            for idx, (c, a, b) in enumerate(segs):
                last_half[0 if a < 512 else 1] = idx
            with tc.tile_pool(name="pb", bufs=1) as pb, \
                 tc.tile_pool(name="pb2", bufs=2) as pb2, \
                 tc.tile_pool(name="pb3", bufs=1) as pb3, \
                 tc.tile_pool(name="pb_ps", bufs=1, space="PSUM") as pb_ps:
                for pair in range(HL // 2):
                    heads = (2 * pair, 2 * pair + 1)
                    # es[p, c, hl, t] = exp(scores^T) for the head pair
                    es = pb.tile([128, NCH, 2, T], MM_DT, tag="es")
                    av = {hl: pb_ps.tile([KC + 2, T], F32, tag=f"av{hl}",
                                         name=f"av{heads[hl]}")
                          for hl in (0, 1)}
                    pdw6 = pb3.tile([128, NCH, 2 * NB], MM_DT, tag="pdw6")

                    for c in range(NCH):
                        t_lo, q0, w = _chunk_window(c)
                        t0 = 128 * c - WIN
                        for n in range(2):
                            # half-chunk scores tile; the two heads' matmuls
                            # use disjoint PE row groups (rb 0/64). Tags
                            # alternate so scores(c,n+1) overlaps exp(c,n).
                            stn = pb_ps.tile([128, T], F32,
                                             tag=f"st{(2 * c + n) % 2}")
                            for hl in (0, 1):
                                rb = 64 * hl
                                nc.tensor.matmul(
                                    stn[:, 512 * hl:512 * (hl + 1)],
                                    k_sb[rb:rb + 64, pair,
                                         128 * c:128 * (c + 1)],
                                    q_sb[rb:rb + 64, pair,
                                         512 * n:512 * (n + 1)],
                                    start=True, stop=True)
                            if EN_BANDK:
                                for a, b in _half_segments(c, n):
                                    for hl in (0, 1):
                                        h = heads[hl]
                                        sl = slice(512 * hl + a - 512 * n,
                                                   512 * hl + b - 512 * n)
                                        nc.vector.tensor_tensor(
                                            stn[:, sl], stn[:, sl],
                                            wt6[:, c, a - t0:b - t0, h],
                                            ALU.add)
                            # softmax numerator, both heads' halves in one op
                            nc.scalar.activation(
                                es[:, c, :, 512 * n:512 * (n + 1)],
                                stn[:].rearrange("p (hl t) -> p hl t", hl=2),
                                AF.Exp)
                            # A @ V (+ ones column -> row 64 = denominator)
                            for hl in (0, 1):
                                nc.tensor.matmul(
                                    av[hl][:, 512 * n:512 * (n + 1)],
                                    vt[heads[hl]][:, c, :],
                                    es[:, c, hl, 512 * n:512 * (n + 1)],
                                    start=(c == 0), stop=False,
                                    skip_group_check=True)
                        if EN_BANDV:
                            # stage both heads' es windows (544B runs) and
                            # read back the compact diagonals (36B runs):
                            # pdw6[p, c, 9*hl+i] = es_hl[p, t0 + p + i]
                            buf = w2[pair * NCH + c].ap()
                            nc.gpsimd.dma_start(
                                _raw(buf, 8 * W2_COLS + q0,
                                     [[W2_COLS, 128], [W2_REG, 2], [1, w]]),
                                _f32(es[:, c, :, t_lo:t_lo + w]))
                            nc.gpsimd.dma_start(
                                pdw6[:, c, :],
                                _mm(_raw(buf, 8 * W2_COLS,
                                         [[W2_COLS + 1, 128], [W2_REG, 2],
                                          [1, NB]])))

                    # start the reciprocal chain as soon as the last A@V
                    # lands: the denominator row (64) is untouched by the
                    # band matmuls, so only the final multiply must wait.
                    rlrs = {}
                    if EN_NORM:
                        for hl in (0, 1):
                            ll = pb3.tile([1, T], F32, tag=f"ll{hl}")
                            nc.vector.tensor_copy(ll[:],
                                                  av[hl][KC:KC + 1, :])
                            lr8 = pb3.tile([128, 8], F32, tag=f"lr8{hl}")
                            nc.scalar.dma_start(
                                lr8[:],
                                ll[:].rearrange("o (p k) -> o p k", p=128))
                            lr8r = pb3.tile([128, 8], F32, tag=f"lr8r{hl}")
                            nc.vector.reciprocal(lr8r[:], lr8[:])
                            rl = pb3.tile([1, T], F32, tag=f"rl{hl}")
                            nc.scalar.dma_start(
                                rl[:].rearrange("o (p k) -> o p k", p=128),
                                lr8r[:])
                            rlr = pb3.tile([KC, T], F32, tag=f"rlr{hl}")
                            nc.gpsimd.partition_broadcast(rlr[:], rl[:])
                            rlrs[hl] = rlr

                    if EN_BANDV:
                        # transpose + row-permute all chunks' pdw into
                        # pm[2i+hl, 128c+p] = pdw6[p, c, 9hl+i] (reuses the
                        # st PSUM banks after the last exp: raw transposes in
                        # the st0 tile, permuted rows in the st1 tile).
                        pmt1 = pb_ps.tile([128, T], F32, tag="st0",
                                          name=f"pmt1_{pair}")
                        pmt2 = pb_ps.tile([128, T], F32, tag="st1",
                                          name=f"pmt2_{pair}")
                        tpsb = pb3.tile([2 * NB, NCH, 128], MM_DT, tag="tpsb")
                        for c in range(NCH):
                            nc.tensor.matmul(
                                pmt1[0:2 * NB, 128 * c:128 * (c + 1)],
                                pdw6[:, c, :],
                                i128_sb[:],
                                start=True, stop=True)
                            nc.vector.tensor_copy(
                                tpsb[:, c, :],
                                pmt1[0:2 * NB, 128 * c:128 * (c + 1)])
                            nc.tensor.matmul(
                                pmt2[0:2 * NB, 128 * c:128 * (c + 1)],
                                perm_sb[:],
                                tpsb[:, c, :],
                                start=True, stop=True)
                        # PSUM reads need 32-aligned partition bases; stage
                        # in SBUF before the shear.
                        pmsb = pb3.tile([2 * NB, T], F32, tag="pmsb")
                        nc.vector.tensor_copy(pmsb[:], pmt2[0:2 * NB, 0:T])
                        # shear-align: pdc[2i+hl, c, i+p] = pm[2i+hl, 128c+p]
                        # (SBUF->SBUF DMAs: engines need aligned partition
                        # bases, DMA does not)
                        pdc = pb2.tile([2 * NB, NCH, WQ], MM_DT, tag="pdc")
                        nc.gpsimd.dma_start(pdc[:], _mm(z18d.ap()))
                        for i in range(NB):
                            eng = (nc.sync, nc.scalar, nc.gpsimd)[i % 3]
                            eng.dma_start(
                                pdc[2 * i:2 * i + 2, :, i:i + 128],
                                _mm(pmsb[2 * i:2 * i + 2, :].rearrange(
                                    "r (c p) -> r c p", c=NCH)))
                        # band_v: av[d, t] += sum_i ev[8-i, d] * pdc[2i+hl, t]
                        for hl in (0, 1):
                            for idx, (c, a, b) in enumerate(segs):
                                t0 = 128 * c - WIN
                                nc.tensor.matmul(
                                    av[hl][0:KC, a:b],
                                    ev18_sb[:, KC * hl:KC * (hl + 1)],
                                    pdc[:, c, a - t0:b - t0],
                                    start=False,
                                    stop=(idx == last_half[0 if a < 512
                                                           else 1]),
                                    skip_group_check=True)

                    # normalize by the denominator row and merge heads
                    for hl in (0, 1):
                        rows = 64 * hl
                        if EN_NORM:
                            nc.vector.tensor_tensor(
                                merged[rows:rows + KC, pair, :],
                                av[hl][0:KC, :], rlrs[hl][:], ALU.mult)
                        else:
                            nc.vector.tensor_copy(
                                merged[rows:rows + KC, pair, :],
                                av[hl][0:KC, :])

            # ---------------- Phase C: output projection ----------------
            with tc.tile_pool(name="pc", bufs=3) as pc, \
                 tc.tile_pool(name="pc_ps", bufs=3, space="PSUM") as pc_ps:
                for m in range(NSUB):
                    for n in range(2):
                        ps = pc_ps.tile([128, 512], F32, tag="o_ps")
                        for k in range(LSUB):
                            nc.tensor.matmul(
                                ps[:],
                                wo_sb[:, k, 128 * m:128 * (m + 1)],
                                merged[:, k, 512 * n:512 * (n + 1)],
                                start=(k == 0), stop=(k == LSUB - 1))
                        ot = pc.tile([128, 512], F32, tag="o_sb")
                        if (2 * m + n) % 2 == 0:
                            nc.vector.tensor_copy(ot[:], ps[:])
                        else:
                            nc.scalar.activation(ot[:], ps[:], AF.Identity)
                        nc.sync.dma_start(
                            outp.ap()[128 * m:128 * (m + 1),
                                      512 * n:512 * (n + 1)],
                            ot[:])

    nc.compile()
    return nc


_CACHE = {}


def _get_program():
    if "nc" not in _CACHE:
        _CACHE["nc"] = build_program()
    return _CACHE["nc"]


def _prep_core_inputs(core, x, c, Wq, bq, Wk, bk, Wv, bv, Wo,
                      emb_rel_k, emb_rel_v, zeros_rk, zeros_w2):
    b, hg = core // 2, core % 2
    hsl = slice(hg * CL, (hg + 1) * CL)
    scale = KC ** -0.5
    ek = np.ascontiguousarray(emb_rel_k[0])  # [9, 64]
    ekt = np.ascontiguousarray(ek.T)         # [64, 9]
    ev = np.ascontiguousarray(emb_rel_v[0])  # [9, 64]
    ekt18 = np.zeros((128, 2 * NB), np.float32)
    ekt18[0:KC, 0:NB] = ekt
    ekt18[KC:128, NB:2 * NB] = ekt
    ev18 = np.zeros((2 * NB, 128), np.float32)
    perm18 = np.zeros((2 * NB, 2 * NB), np.float32)
    for i in range(NB):
        for hl in range(2):
            ev18[2 * i + hl, KC * hl:KC * (hl + 1)] = ev[NB - 1 - i]
            # out row 2i+hl <- transposed row 9*hl+i
            perm18[NB * hl + i, 2 * i + hl] = 1.0
    ins = {
        "ones8": np.concatenate([np.ones((128, NCH, 1), np.float32),
                                 np.zeros((128, NCH, 1), np.float32)], axis=2),
        "xb": np.ascontiguousarray(x[b]),
        "cb": np.ascontiguousarray(c[b]),
        "wqt": np.ascontiguousarray((Wq[hsl] * scale).T),
        "wkt": np.ascontiguousarray(Wk[hsl].T),
        "wvt": np.ascontiguousarray(Wv[hsl].T),
        "wot": np.ascontiguousarray(Wo[:, hsl].T),
        "bq2": np.ascontiguousarray((bq[hsl] * scale).reshape(LSUB, 128).T),
        "bk2": np.ascontiguousarray(bk[hsl].reshape(LSUB, 128).T),
        "bvr": np.ascontiguousarray(np.tile(bv[hsl][None, :], (128, 1))),
        "ekt18": ekt18,
        "ev18": ev18,
        "i128": np.eye(128, dtype=np.float32),
        "perm18": perm18,
        "z18": np.zeros((2 * NB, NCH, WQ), np.float32),
    }
    for ch in range(NCH):
        ins[f"rk{ch}"] = zeros_rk
    for i in range((HL // 2) * NCH):
        ins[f"w2_{i}"] = zeros_w2
    return ins


def kernel(**inputs):
    inputs = {k: np.asarray(v, dtype=np.float32) for k, v in inputs.items()}
    nc = _get_program()
    zeros_rk = np.zeros(RK_LEN, np.float32)
    zeros_w2 = np.zeros(2 * W2_REG, np.float32)
    in_maps = [
        _prep_core_inputs(
            core, inputs["x"], inputs["c"],
            inputs["Wq"], inputs["bq"], inputs["Wk"], inputs["bk"],
            inputs["Wv"], inputs["bv"], inputs["Wo"],
            inputs["emb_rel_k"], inputs["emb_rel_v"],
            zeros_rk, zeros_w2)
        for core in range(8)
    ]
    res = run_bass_kernel_spmd(nc, in_maps, core_ids=list(range(8)),
                               **_CACHE.get("run_kwargs", {}))
    _CACHE["last_result"] = res
    parts = [r["outp"] for r in res.results]
    bo = inputs["bo"]
    out = np.stack([parts[2 * b] + parts[2 * b + 1] + bo[:, None]
                    for b in range(B)])
    return out.astype(np.float32)


# revision 31
# speedup vs baseline: 1.0221x; 1.0221x over previous
"""Trainium2 Bass kernel for nn_MultiHeadAttention_42752104464925.

Multi-head attention (VITS-style) with windowed relative position embeddings
(window=4, heads_share=True).

Math notes
----------
With L=1024, WIN=4, the relative-key logits term rel_to_abs(q_scaled @ rel_k^T)
is a 9-diagonal band:   scores[t,s] += q_scaled[t] . emb_k[s-t+4]   (|s-t|<=4)
and the relative-value term is:
  out[t] += sum_j p[t, t+j-4] * emb_v[j]   (0 <= t+j-4 < L)

Sharding: 8 cores = 4 batches x 2 head-groups (6 heads each). Each core
computes QKV for its 384 channels, attention for its 6 heads, and a partial
output projection Wo[:, slice] @ merged. Host sums the two partials per batch.

Device layout per head: scores are computed TRANSPOSED (S^T[s,t], s on
partitions) so that A@V needs no transpose of the attention weights; the
softmax denominator L[t] (a partition-dim sum) comes for free from a ones
column appended to the V^T stationary operand (M=66).

Band application (the shear between diagonal and absolute coordinates) is
DMA-descriptor-bound if done naively, so both bands are engineered for
coarse descriptor runs:

* band_k: Rt[t,j] values for all 6 heads are written head-interleaved into a
  per-chunk skewed DRAM region (24B runs, one per (t,j)), laid out so the
  readback of the full [128, 136] band window for ALL 6 heads is a single
  per-partition-contiguous read (3264B runs). The zero padding the windows
  rely on is supplied as host-side zero input tensors.
* band_v: the 9 diagonals of exp(S) are read back from the es window staging
  as a compact [128, 9]-per-head tile (36B contiguous runs), transposed and
  row-permuted on TensorE, shear-aligned with 9 SBUF->SBUF DMAs, and folded
  into the A@V PSUM accumulation with small K=18 matmuls against a
  host-precomputed embedding matrix.

Scheduling: scores/exp are double-buffered at 512-column granularity (two
half-chunk PSUM tiles alternate) so TensorE and ScalarE pipeline; DMAs are
spread across the sync/scalar HWDGE queues and the gpsimd SWDGE queue to
avoid serializing on one sequencer.
"""

import math

import numpy as np

import concourse.bacc as bacc
import concourse.bass as bass
import concourse.mybir as mybir
import concourse.tile as tile
from concourse import library_config
from concourse.bass_utils import run_bass_kernel_spmd

# Problem constants (hardcoded per harness contract).
B, C, T, H, KC, WIN = 4, 768, 1024, 12, 64, 4
HL = 6            # heads per core
CL = HL * KC      # 384 local channels
NSUB = C // 128   # 6 k-subtiles over C
LSUB = CL // 128  # 3 subtiles over CL
NCH = T // 128    # 8 s-chunks
NB = 9            # band width (2*WIN+1)
WQ = 136          # band window width per 128-chunk (128 + 2*WIN)
# band_k skewed staging: region element addr = RK_GUARD + RK_ROW*p + 6*f + h
# holds W[p, f, h] = Rt_h[t0+f, p-f+8]; rows are 816 elements used of RK_ROW.
RK_ROW = 6 * 144
RK_GUARD = 6912
RK_LEN = RK_GUARD + RK_ROW * WQ
W2_ROWS, W2_COLS = 144, 136  # padded expS window staging (8 zero rows each end)
W2_REG = W2_ROWS * W2_COLS   # one head's region; a pair shares one tensor

F32 = mybir.dt.float32
AF = mybir.ActivationFunctionType
ALU = mybir.AluOpType

# Matmul input dtype. float32r streams at full PE rate (1 cyc/row for N>=256)
# with fp32 storage; plain float32 runs at 1/4 rate.
MM_DT = mybir.dt.float32r

EN_BANDK = True    # band_k window read + add into scores
EN_BANDV = True    # band_v pdw gather + transpose + matmul accumulation
EN_NORM = True     # softmax normalization


def _mm(x):
    return x if MM_DT == F32 else x.bitcast(MM_DT)


def _f32(x):
    return x if MM_DT == F32 else x.bitcast(F32)


def _raw(t_ap, off, dims):
    """Raw element-offset AP into (the tensor behind) an AP."""
    return bass.AP(tensor=t_ap.tensor, offset=t_ap.offset + off, ap=dims)


def _chunk_window(c):
    """Clipped t-window [t_lo, t_lo+w) for s-chunk c; q0 = offset into the
    unclipped 136-wide window starting at t0 = 128c - 4."""
    t0 = 128 * c - WIN
    t_lo = max(0, t0)
    q0 = t_lo - t0
    w = min(T, t0 + WQ) - t_lo
    return t_lo, q0, w


def _half_segments(c, n):
    """Absolute-t segments of chunk c's band window inside half n."""
    t_lo, q0, w = _chunk_window(c)
    a = max(t_lo, 512 * n)
    b = min(t_lo + w, 512 * (n + 1))
    return [(a, b)] if a < b else []


def _bandv_segments():
    """(c, a, b) absolute-t segments of each chunk's band window, split at
    PSUM bank (512) boundaries."""
    segs = []
    for c in range(NCH):
        t0 = 128 * c - WIN
        a, b = max(t0, 0), min(t0 + WQ, T)
        cuts = [a] + [x for x in (512,) if a < x < b] + [b]
        for k in range(len(cuts) - 1):
            segs.append((c, cuts[k], cuts[k + 1]))
    return segs


def build_program():
    nc = bacc.Bacc("TRN2", target_bir_lowering=False, debug=False,
                   enable_asserts=True)

    # ---- I/O ----
    xb = nc.dram_tensor("xb", [C, T], F32, kind="ExternalInput")
    cb = nc.dram_tensor("cb", [C, T], F32, kind="ExternalInput")
    wqt = nc.dram_tensor("wqt", [C, CL], F32, kind="ExternalInput")
    wkt = nc.dram_tensor("wkt", [C, CL], F32, kind="ExternalInput")
    wvt = nc.dram_tensor("wvt", [C, CL], F32, kind="ExternalInput")
    wot = nc.dram_tensor("wot", [CL, C], F32, kind="ExternalInput")
    bq2 = nc.dram_tensor("bq2", [128, LSUB], F32, kind="ExternalInput")
    bk2 = nc.dram_tensor("bk2", [128, LSUB], F32, kind="ExternalInput")
    bvr = nc.dram_tensor("bvr", [128, CL], F32, kind="ExternalInput")
    ekt18d = nc.dram_tensor("ekt18", [128, 2 * NB], F32, kind="ExternalInput")
    ev18d = nc.dram_tensor("ev18", [2 * NB, 128], F32, kind="ExternalInput")
    i128d = nc.dram_tensor("i128", [128, 128], F32, kind="ExternalInput")
    permd = nc.dram_tensor("perm18", [2 * NB, 2 * NB], F32,
                           kind="ExternalInput")
    z18d = nc.dram_tensor("z18", [2 * NB, NCH, WQ], F32,
                          kind="ExternalInput")
    ones8 = nc.dram_tensor("ones8", [128, NCH, 2], F32, kind="ExternalInput")
    # zero-padded staging buffers (host supplies zeros; device writes data)
    rk = [nc.dram_tensor(f"rk{c}", [RK_LEN], F32, kind="ExternalInput")
          for c in range(NCH)]
    w2 = [nc.dram_tensor(f"w2_{i}", [2 * W2_REG], F32, kind="ExternalInput")
          for i in range((HL // 2) * NCH)]
    outp = nc.dram_tensor("outp", [C, T], F32, kind="ExternalOutput")

    with tile.TileContext(nc) as tc:
        nc.gpsimd.load_library(library_config.attn)
        with tc.tile_pool(name="persist", bufs=1) as pp:
            # persistent SBUF
            q_sb = pp.tile([128, LSUB, T], MM_DT, tag="q_sb")
            k_sb = pp.tile([128, LSUB, T], MM_DT, tag="k_sb")
            vt = [pp.tile([128, NCH, KC + 2], MM_DT, tag=f"vt{h}", name=f"vt{h}")
                  for h in range(HL)]
            wo_sb = pp.tile([128, LSUB, C], MM_DT, tag="wo_sb")
            merged = pp.tile([128, LSUB, T], MM_DT, tag="merged")
            ekt_sb = pp.tile([128, 2 * NB], MM_DT, tag="ekt_sb")
            ev18_sb = pp.tile([2 * NB, 128], MM_DT, tag="ev18_sb")
            i128_sb = pp.tile([128, 128], MM_DT, tag="i128_sb")
            perm_sb = pp.tile([2 * NB, 2 * NB], MM_DT, tag="perm_sb")
            bq_sb = pp.tile([128, LSUB], F32, tag="bq_sb")
            bk_sb = pp.tile([128, LSUB], F32, tag="bk_sb")
            bv_sb = pp.tile([128, CL], F32, tag="bv_sb")
            # band_k windows for all chunks/heads: [p, c, f, h]
            wt6 = pp.tile([128, NCH, WQ, HL], F32, tag="wt6")

            # ---------------- Phase A: projections ----------------
            with tc.tile_pool(name="pa", bufs=1) as pa, \
                 tc.tile_pool(name="pa_ps", bufs=3, space="PSUM") as pa_ps, \
                 tc.tile_pool(name="pa_ps2", bufs=2, space="PSUM") as pa_ps2, \
                 tc.tile_pool(name="pa_ps3", bufs=2, space="PSUM") as pa_ps3:
                x_sb = pa.tile([128, NSUB, T], MM_DT, tag="x_sb")
                c_sb = pa.tile([128, NSUB, T], MM_DT, tag="c_sb")
                wq_sb = pa.tile([128, NSUB, CL], MM_DT, tag="wq_sb")
                wk_sb = pa.tile([128, NSUB, CL], MM_DT, tag="wk_sb")
                wv_sb = pa.tile([128, NSUB, CL], MM_DT, tag="wv_sb")
                # Rt staging, head-interleaved: [t_part, c, j, h]
                rts = pa.tile([128, NCH, NB, HL], F32, tag="rts")

                # whole-tensor loads (per-DMA fixed cost dominates small
                # transfers), ordered so the Q matmuls can start earliest
                nc.scalar.dma_start(wq_sb[:], _mm(wqt.ap().rearrange(
                    "(s p) m -> p s m", p=128)))
                nc.sync.dma_start(x_sb[:], _mm(xb.ap().rearrange(
                    "(s p) t -> p s t", p=128)))
                nc.scalar.dma_start(wk_sb[:], _mm(wkt.ap().rearrange(
                    "(s p) m -> p s m", p=128)))
                nc.scalar.dma_start(c_sb[:], _mm(cb.ap().rearrange(
                    "(s p) t -> p s t", p=128)))
                nc.sync.dma_start(wv_sb[:], _mm(wvt.ap().rearrange(
                    "(s p) m -> p s m", p=128)))
                # persist-tile loads on the idle SWDGE queue (keeps the
                # scalar sequencer free for the QK bias-identities)
                nc.gpsimd.dma_start(wo_sb[:], _mm(wot.ap().rearrange(
                    "(s p) m -> p s m", p=128)))
                nc.gpsimd.dma_start(ekt_sb[:], _mm(ekt18d.ap()))
                nc.gpsimd.dma_start(ev18_sb[:], _mm(ev18d.ap()))
                nc.gpsimd.dma_start(i128_sb[:], _mm(i128d.ap()))
                nc.gpsimd.dma_start(perm_sb[:], _mm(permd.ap()))
                nc.gpsimd.dma_start(bq_sb[:], bq2.ap())
                nc.gpsimd.dma_start(bk_sb[:], bk2.ap())
                nc.gpsimd.dma_start(bv_sb[:], bvr.ap())
                for h in range(HL):
                    nc.gpsimd.dma_start(vt[h][:, :, KC:KC + 2],
                                        _mm(ones8.ap()))

                # Q and K: out[dl, t] = sum_c W*T[c, dl] * x[c, t]  (+bias)
                for dst, wsb, src, bias in ((q_sb, wq_sb, x_sb, bq_sb),
                                            (k_sb, wk_sb, c_sb, bk_sb)):
                    for m in range(LSUB):
                        for n in range(2):
                            ps = pa_ps.tile([128, 512], F32, tag="qk_ps")
                            for k in range(NSUB):
                                nc.tensor.matmul(
                                    ps[:],
                                    wsb[:, k, 128 * m:128 * (m + 1)],
                                    src[:, k, 512 * n:512 * (n + 1)],
                                    start=(k == 0), stop=(k == NSUB - 1))
                            # fused copy+bias on ACT (idle in phase A)
                            nc.scalar.activation(
                                dst[:, m, 512 * n:512 * (n + 1)], ps[:],
                                AF.Identity, bias=bias[:, m:m + 1])

                # Rt[t, j] for the head pair of subtile `sub` in one matmul:
                # stationary q-chunk [128, 128], moving block-diagonal
                # ekt18 [128, 18]  ->  out[t, 9*hl + j]
                for sub in range(LSUB):
                    for c in range(NCH):
                        rt_ps = pa_ps3.tile([128, 2 * NB], F32, tag="rt_ps")
                        nc.tensor.matmul(
                            rt_ps[:],
                            q_sb[:, sub, 128 * c:128 * (c + 1)],
                            ekt_sb[:],
                            start=True, stop=True)
                        nc.vector.tensor_copy(
                            rts[:, c, :, 2 * sub:2 * sub + 2].transpose(
                                [0, 2, 1]),
                            rt_ps[:].rearrange("p (hl j) -> p hl j", hl=2))

                # band_k staging: shear-write Rt into per-chunk skewed regions
                # (24B runs), then read each chunk's full 6-head window back
                # with per-partition-contiguous 3264B runs.
                if EN_BANDK:
                    for c in range(NCH):
                        nc.sync.dma_start(
                            _raw(rk[c].ap(), RK_GUARD - 864 * 4 + 24,
                                 [[870, 128], [864, NB], [1, HL]]),
                            rts[:, c, :, :])
                        if c > 0:
                            nc.sync.dma_start(
                                _raw(rk[c].ap(), 0,
                                     [[870, 4], [864, NB], [1, HL]]),
                                rts[124:128, c - 1, :, :])
                        if c < NCH - 1:
                            nc.sync.dma_start(
                                _raw(rk[c].ap(), RK_GUARD + 864 * 124 + 792,
                                     [[870, 4], [864, NB], [1, HL]]),
                                rts[0:4, c + 1, :, :])
                        nc.sync.dma_start(
                            wt6[:, c, :, :],
                            _raw(rk[c].ap(), RK_GUARD,
                                 [[RK_ROW, 128], [1, 6 * WQ]]))

                # V^T: out[s, dl] = sum_c c_b[c, s] * WvT[c, dl] (+bias),
                # written per head into [128, NCH, 66] tiles, col 64 = ones.
                for c in range(NCH):
                    vt_ps = pa_ps2.tile([128, CL], F32, tag="vt_ps")
                    for k in range(NSUB):
                        nc.tensor.matmul(
                            vt_ps[:],
                            c_sb[:, k, 128 * c:128 * (c + 1)],
                            wv_sb[:, k, :],
                            start=(k == 0), stop=(k == NSUB - 1))
                    for h in range(HL):
                        nc.vector.tensor_tensor(
                            vt[h][:, c, 0:KC], vt_ps[:, KC * h:KC * (h + 1)],
                            bv_sb[:, KC * h:KC * (h + 1)], ALU.add)

            # ---------------- Phase B: attention ----------------
            segs = _bandv_segments()
            last_half = {}
            for idx, (c, a, b) in enumerate(segs):
                last_half[0 if a < 512 else 1] = idx
            with tc.tile_pool(name="pb", bufs=1) as pb, \
                 tc.tile_pool(name="pb2", bufs=2) as pb2, \
                 tc.tile_pool(name="pb3", bufs=1) as pb3, \
                 tc.tile_pool(name="pb_ps", bufs=1, space="PSUM") as pb_ps:
                for pair in range(HL // 2):
                    heads = (2 * pair, 2 * pair + 1)
                    # es[p, c, hl, t] = exp(scores^T) for the head pair
                    es = pb.tile([128, NCH, 2, T], MM_DT, tag="es")
                    av = {hl: pb_ps.tile([KC + 2, T], F32, tag=f"av{hl}",
                                         name=f"av{heads[hl]}")
                          for hl in (0, 1)}
                    pdw6 = pb2.tile([128, NCH, 2 * NB], MM_DT, tag="pdw6")
                    # zero-fill the shear target early (no dependencies)
                    pdc = pb2.tile([2 * NB, NCH, WQ], MM_DT, tag="pdc")
                    if EN_BANDV:
                        nc.gpsimd.dma_start(pdc[:], _mm(z18d.ap()))

                    for c in range(NCH):
                        t_lo, q0, w = _chunk_window(c)
                        t0 = 128 * c - WIN
                        for n in range(2):
                            # half-chunk scores tile; the two heads' matmuls
                            # use disjoint PE row groups (rb 0/64). Tags
                            # alternate so scores(c,n+1) overlaps exp(c,n).
                            stn = pb_ps.tile([128, T], F32,
                                             tag=f"st{(2 * c + n) % 2}")
                            for hl in (0, 1):
                                rb = 64 * hl
                                nc.tensor.matmul(
                                    stn[:, 512 * hl:512 * (hl + 1)],
                                    k_sb[rb:rb + 64, pair,
                                         128 * c:128 * (c + 1)],
                                    q_sb[rb:rb + 64, pair,
                                         512 * n:512 * (n + 1)],
                                    start=True, stop=True)
                            if EN_BANDK:
                                for a, b in _half_segments(c, n):
                                    for hl in (0, 1):
                                        h = heads[hl]
                                        sl = slice(512 * hl + a - 512 * n,
                                                   512 * hl + b - 512 * n)
                                        nc.vector.tensor_tensor(
                                            stn[:, sl], stn[:, sl],
                                            wt6[:, c, a - t0:b - t0, h],
                                            ALU.add)
                            # softmax numerator, both heads' halves in one op
                            nc.scalar.activation(
                                es[:, c, :, 512 * n:512 * (n + 1)],
                                stn[:].rearrange("p (hl t) -> p hl t", hl=2),
                                AF.Exp)
                            # A @ V (+ ones column -> row 64 = denominator)
                            for hl in (0, 1):
                                nc.tensor.matmul(
                                    av[hl][:, 512 * n:512 * (n + 1)],
                                    vt[heads[hl]][:, c, :],
                                    es[:, c, hl, 512 * n:512 * (n + 1)],
                                    start=(c == 0), stop=False,
                                    skip_group_check=True)
                        if EN_BANDV:
                            # stage both heads' es windows (544B runs) and
                            # read back the compact diagonals (36B runs):
                            # pdw6[p, c, 9*hl+i] = es_hl[p, t0 + p + i]
                            buf = w2[pair * NCH + c].ap()
                            nc.gpsimd.dma_start(
                                _raw(buf, 8 * W2_COLS + q0,
                                     [[W2_COLS, 128], [W2_REG, 2], [1, w]]),
                                _f32(es[:, c, :, t_lo:t_lo + w]))
                            nc.gpsimd.dma_start(
                                pdw6[:, c, :],
                                _mm(_raw(buf, 8 * W2_COLS,
                                         [[W2_COLS + 1, 128], [W2_REG, 2],
                                          [1, NB]])))

                    # start the reciprocal chain as soon as the last A@V
                    # lands: the denominator row (64) is untouched by the
                    # band matmuls, so only the final multiply must wait.
                    rlrs = {}
                    if EN_NORM:
                        for hl in (0, 1):
                            ll = pb3.tile([1, T], F32, tag=f"ll{hl}")
                            nc.vector.tensor_copy(ll[:],
                                                  av[hl][KC:KC + 1, :])
                            lr8 = pb3.tile([128, 8], F32, tag=f"lr8{hl}")
                            nc.scalar.dma_start(
                                lr8[:],
                                ll[:].rearrange("o (p k) -> o p k", p=128))
                            lr8r = pb3.tile([128, 8], F32, tag=f"lr8r{hl}")
                            nc.vector.reciprocal(lr8r[:], lr8[:])
                            rl = pb3.tile([1, T], F32, tag=f"rl{hl}")
                            nc.scalar.dma_start(
                                rl[:].rearrange("o (p k) -> o p k", p=128),
                                lr8r[:])
                            rlr = pb3.tile([KC, T], F32, tag=f"rlr{hl}")
                            nc.gpsimd.partition_broadcast(rlr[:], rl[:])
                            rlrs[hl] = rlr

                    if EN_BANDV:
                        # transpose + row-permute all chunks' pdw into
                        # pm[2i+hl, 128c+p] = pdw6[p, c, 9hl+i] (reuses the
                        # st PSUM banks after the last exp: raw transposes in
                        # the st0 tile, permuted rows in the st1 tile).
                        pmt1 = pb_ps.tile([128, T], F32, tag="st0",
                                          name=f"pmt1_{pair}")
                        pmt2 = pb_ps.tile([128, T], F32, tag="st1",
                                          name=f"pmt2_{pair}")
                        tpsb = pb3.tile([2 * NB, NCH, 128], MM_DT, tag="tpsb")
                        for c in range(NCH):
                            nc.tensor.matmul(
                                pmt1[0:2 * NB, 128 * c:128 * (c + 1)],
                                pdw6[:, c, :],
                                i128_sb[:],
                                start=True, stop=True)
                            nc.vector.tensor_copy(
                                tpsb[:, c, :],
                                pmt1[0:2 * NB, 128 * c:128 * (c + 1)])
                            nc.tensor.matmul(
                                pmt2[0:2 * NB, 128 * c:128 * (c + 1)],
                                perm_sb[:],
                                tpsb[:, c, :],
                                start=True, stop=True)
                        # PSUM reads need 32-aligned partition bases; stage
                        # in SBUF before the shear.
                        pmsb = pb3.tile([2 * NB, T], F32, tag="pmsb")
                        nc.vector.tensor_copy(pmsb[:], pmt2[0:2 * NB, 0:T])
                        # shear-align: pdc[2i+hl, c, i+p] = pm[2i+hl, 128c+p]
                        # (SBUF->SBUF DMAs: engines need aligned partition
                        # bases, DMA does not)
                        for i in range(NB):
                            eng = (nc.sync, nc.scalar, nc.gpsimd)[i % 3]
                            eng.dma_start(
                                pdc[2 * i:2 * i + 2, :, i:i + 128],
                                _mm(pmsb[2 * i:2 * i + 2, :].rearrange(
                                    "r (c p) -> r c p", c=NCH)))
                        # band_v: av[d, t] += sum_i ev[8-i, d] * pdc[2i+hl, t]
                        for hl in (0, 1):
                            for idx, (c, a, b) in enumerate(segs):
                                t0 = 128 * c - WIN
                                nc.tensor.matmul(
                                    av[hl][0:KC, a:b],
                                    ev18_sb[:, KC * hl:KC * (hl + 1)],
                                    pdc[:, c, a - t0:b - t0],
                                    start=False,
                                    stop=(idx == last_half[0 if a < 512
                                                           else 1]),
                                    skip_group_check=True)

                    # normalize by the denominator row and merge heads
                    for hl in (0, 1):
                        rows = 64 * hl
                        if EN_NORM:
                            nc.vector.tensor_tensor(
                                merged[rows:rows + KC, pair, :],
                                av[hl][0:KC, :], rlrs[hl][:], ALU.mult)
                        else:
                            nc.vector.tensor_copy(
                                merged[rows:rows + KC, pair, :],
                                av[hl][0:KC, :])

                # ------------ Phase C: output projection ------------
                # (same PSUM pool: o_ps reuses the st banks so the first
                # m-tiles overlap the last pair's band/normalize tail)
                with tc.tile_pool(name="pc", bufs=3) as pc:
                    for m in range(NSUB):
                        ps = pb_ps.tile([128, T], F32, tag=f"st{m % 2}",
                                        name=f"o_ps{m}")
                        for n in range(2):
                            for k in range(LSUB):
                                nc.tensor.matmul(
                                    ps[:, 512 * n:512 * (n + 1)],
                                    wo_sb[:, k, 128 * m:128 * (m + 1)],
                                    merged[:, k, 512 * n:512 * (n + 1)],
                                    start=(k == 0), stop=(k == LSUB - 1))
                        for n in range(2):
                            ot = pc.tile([128, 512], F32, tag="o_sb")
                            if (2 * m + n) % 2 == 0:
                                nc.vector.tensor_copy(
                                    ot[:], ps[:, 512 * n:512 * (n + 1)])
                            else:
                                nc.scalar.activation(
                                    ot[:], ps[:, 512 * n:512 * (n + 1)],
                                    AF.Identity)
                            eng = nc.sync if n == 0 else nc.gpsimd
                            eng.dma_start(
                                outp.ap()[128 * m:128 * (m + 1),
                                          512 * n:512 * (n + 1)],
                                ot[:])

    nc.compile()
    return nc


_CACHE = {}


def _get_program():
    if "nc" not in _CACHE:
        _CACHE["nc"] = build_program()
    return _CACHE["nc"]


def _prep_core_inputs(core, x, c, Wq, bq, Wk, bk, Wv, bv, Wo,
                      emb_rel_k, emb_rel_v, zeros_rk, zeros_w2):
    b, hg = core // 2, core % 2
    hsl = slice(hg * CL, (hg + 1) * CL)
    scale = KC ** -0.5
    ek = np.ascontiguousarray(emb_rel_k[0])  # [9, 64]
    ekt = np.ascontiguousarray(ek.T)         # [64, 9]
    ev = np.ascontiguousarray(emb_rel_v[0])  # [9, 64]
    ekt18 = np.zeros((128, 2 * NB), np.float32)
    ekt18[0:KC, 0:NB] = ekt
    ekt18[KC:128, NB:2 * NB] = ekt
    ev18 = np.zeros((2 * NB, 128), np.float32)
    perm18 = np.zeros((2 * NB, 2 * NB), np.float32)
    for i in range(NB):
        for hl in range(2):
            ev18[2 * i + hl, KC * hl:KC * (hl + 1)] = ev[NB - 1 - i]
            # out row 2i+hl <- transposed row 9*hl+i
            perm18[NB * hl + i, 2 * i + hl] = 1.0
    ins = {
        "ones8": np.concatenate([np.ones((128, NCH, 1), np.float32),
                                 np.zeros((128, NCH, 1), np.float32)], axis=2),
        "xb": np.ascontiguousarray(x[b]),
        "cb": np.ascontiguousarray(c[b]),
        "wqt": np.ascontiguousarray((Wq[hsl] * scale).T),
        "wkt": np.ascontiguousarray(Wk[hsl].T),
        "wvt": np.ascontiguousarray(Wv[hsl].T),
        "wot": np.ascontiguousarray(Wo[:, hsl].T),
        "bq2": np.ascontiguousarray((bq[hsl] * scale).reshape(LSUB, 128).T),
        "bk2": np.ascontiguousarray(bk[hsl].reshape(LSUB, 128).T),
        "bvr": np.ascontiguousarray(np.tile(bv[hsl][None, :], (128, 1))),
        "ekt18": ekt18,
        "ev18": ev18,
        "i128": np.eye(128, dtype=np.float32),
        "perm18": perm18,
        "z18": np.zeros((2 * NB, NCH, WQ), np.float32),
    }
    for ch in range(NCH):
        ins[f"rk{ch}"] = zeros_rk
    for i in range((HL // 2) * NCH):
        ins[f"w2_{i}"] = zeros_w2
    return ins


def kernel(**inputs):
    inputs = {k: np.asarray(v, dtype=np.float32) for k, v in inputs.items()}
    nc = _get_program()
    zeros_rk = np.zeros(RK_LEN, np.float32)
    zeros_w2 = np.zeros(2 * W2_REG, np.float32)
    in_maps = [
        _prep_core_inputs(
            core, inputs["x"], inputs["c"],
            inputs["Wq"], inputs["bq"], inputs["Wk"], inputs["bk"],
            inputs["Wv"], inputs["bv"], inputs["Wo"],
            inputs["emb_rel_k"], inputs["emb_rel_v"],
            zeros_rk, zeros_w2)
        for core in range(8)
    ]
    res = run_bass_kernel_spmd(nc, in_maps, core_ids=list(range(8)),
                               **_CACHE.get("run_kwargs", {}))
    _CACHE["last_result"] = res
    parts = [r["outp"] for r in res.results]
    bo = inputs["bo"]
    out = np.stack([parts[2 * b] + parts[2 * b + 1] + bo[:, None]
                    for b in range(B)])
    return out.astype(np.float32)
